# revision 93
# baseline (speedup 1.0000x reference)
"""Trainium2 Bass kernel for nn_CGLayer (gnn_message_passing).

Contract: kernel(**inputs) takes FULL inputs (as reference.setup_inputs()),
returns FULL output [8,128,1,16,9] f32. Internally: data-parallel over the
batch dim across 8 NeuronCores; per core one batch element.

Algebraic reduction (exact):
  X   = conn @ vertices                  (message passing, per batch)
  Y   = mix_nl(cg(X, X))                 (per-node quadratic in X)
  S   = sum_j sph[:, j, :]               (host-computed; neighbor sum
  Z   = mix_rel(cg(Y, S))                 commutes through the rel-CG)
  out = Z / sqrt(sum Z^2 / 16)           (host epilogue, global)

Device pipeline per core (bf16 hot path, ~23us TimelineSim vs 134us for
the fp32 feature-partition baseline):
  A:  X[i,144] f32   = matmul(lhsT=connT, rhs=vcat)
  B:  P[i,9984] bf16 = 13 DVE/Pool stride-tricked pair-product ops over a
                       dense symmetry-folded slot layout (m1<=m2 for l1=l2).
      Chunk transposes to slot-partition layout run on two lanes:
       - 54 chunks via XBAR dma_start_transpose (7 instrs, SBUF->SBUF,
         no PE / no PSUM copies; ops contiguous in slot space share one
         instruction)
       - 24 chunks via PE transpose -> PSUM -> batched Act copies
      78 PE bf16 mix matmuls accumulate (start=False) into bank-pre-zeroed
      ymix tiles (a-major rows; PSUM zero-region = whole 2KB bank, so
      dummy start=True matmuls pre-zero and groups never share wipes)
  C:  one ysbA/ysbBC copy pair, then 9 chains x 4 accumulating matmuls
      (K=96/32/16/16) into psnA/psnB; two 3D DVE scale ops multiply by
      ssum[:, n] into T2 bf16; two DMAs ship T2 out
Host: n-sum of T2, unpack e=(l,c',k), global per-l normalization.
"""
import numpy as np
from math import factorial, sqrt

MAXL = 2
CH = 16
NN = 128
NB = 8
LDIM = [1, 3, 5]
FOFF = [0, 16, 64]
NF = 144
SG_NCOL = [16, 32, 48, 32, 16]

# ------------------------------------------------------------- CG tables
def _cg_coeff(j1, m1, j2, m2, j3, m3):
    if m3 != m1 + m2:
        return 0.0
    pre = sqrt((2 * j3 + 1) * factorial(j3 + j1 - j2) * factorial(j3 - j1 + j2)
               * factorial(j1 + j2 - j3) / factorial(j1 + j2 + j3 + 1))
    pre *= sqrt(factorial(j3 + m3) * factorial(j3 - m3) * factorial(j1 - m1)
                * factorial(j1 + m1) * factorial(j2 - m2) * factorial(j2 + m2))
    s = 0.0
    vmin = max(0, j2 - j3 - m1, j1 - j3 + m2)
    vmax = min(j1 + j2 - j3, j1 - m1, j2 + m2)
    for v in range(vmin, vmax + 1):
        s += (-1) ** v / (factorial(v) * factorial(j1 + j2 - j3 - v)
                          * factorial(j1 - m1 - v) * factorial(j2 + m2 - v)
                          * factorial(j3 - j2 + m1 + v) * factorial(j3 - j1 - m2 + v))
    return pre * s


def _cg_matrix(l1, l2, l):
    M = np.zeros((2 * l1 + 1, 2 * l2 + 1, 2 * l + 1))
    for m1 in range(-l1, l1 + 1):
        for m2 in range(-l2, l2 + 1):
            if -l <= m1 + m2 <= l:
                M[m1 + l1, m2 + l2, m1 + m2 + l] = _cg_coeff(l1, m1, l2, m2, l, m1 + m2)
    return M


def _valid_pairs(l):
    return [(l1, l2) for l1 in range(3) for l2 in range(3)
            if abs(l1 - l2) <= l <= l1 + l2]

# ---------------------------------------------- dense symmetric slot layout
def _product_ops():
    ops = []
    qoff = 0
    for l1 in range(3):
        for l2 in range(l1, 3):
            for m1 in range(2 * l1 + 1):
                mt1 = m1 - l1
                m2_lo = max(0, l2 - 2 - mt1)
                m2_hi = min(2 * l2, l2 + 2 - mt1)
                if l1 == l2:
                    m2_lo = max(m2_lo, m1)
                nm2 = m2_hi - m2_lo + 1
                if nm2 <= 0:
                    continue
                ops.append(dict(l1=l1, l2=l2, m1=m1, m2_lo=m2_lo, nm2=nm2,
                                qoff=qoff))
                qoff += nm2 * 256
    return ops, qoff

PRODUCT_OPS, NSLOT = _product_ops()          # 13 ops, 9984 slots
NCHUNK = NSLOT // 128                        # 78


def _chunk_groups():
    gs = []
    for op in PRODUCT_OPS:
        mt1 = op["m1"] - op["l1"]
        for m2i in range(op["nm2"]):
            g = mt1 + (op["m2_lo"] + m2i - op["l2"]) + 2
            gs.extend([g, g])
    return gs

CHUNK_G = _chunk_groups()
assert len(CHUNK_G) == NCHUNK
W2COFF = np.concatenate([[0], np.cumsum([SG_NCOL[g] for g in CHUNK_G])])
W2COLS = int(W2COFF[-1])                     # 2464

_QIDX = {(op["l1"], op["l2"], op["m1"]): i for i, op in enumerate(PRODUCT_OPS)}
_CAR, _DAR = np.meshgrid(np.arange(16), np.arange(16), indexing="ij")

# engine/lane assignment per product op (key = (l1,l2,m1)):
#   products: Pool ~3.3k of 9984 elems (2.06 ns/elem), DVE the rest (1.09)
#   transposes: XBAR DMA lane for 48 chunks, PE lane for 30
_POOL_OPS = {(0, 2, 0), (2, 2, 0), (2, 2, 2), (2, 2, 3)}
# XBAR transpose instruction groups (ops contiguous in slot space can share
# one instruction); everything else (Pool's 4 ops, 24 chunks) goes PE-lane
_XBAR_GROUPS = [[(1, 2, 0)], [(1, 2, 1)],
                [(1, 1, 0), (1, 1, 1), (1, 1, 2)], [(1, 2, 2)],
                [(2, 2, 1)], [(0, 1, 0)], [(0, 0, 0)]]
_DMA_T_OPS = {k for grp in _XBAR_GROUPS for k in grp}
# issue order: XBAR-feeding DVE ops early, ordered so XBAR groups complete
# products in dispatch order; Pool ops interleave
_OP_ORDER = [(1, 2, 0), (0, 2, 0), (1, 2, 1), (2, 2, 0), (1, 1, 0),
             (1, 1, 1), (1, 1, 2), (2, 2, 2), (1, 2, 2), (2, 2, 3),
             (2, 2, 1), (0, 1, 0), (0, 0, 0)]

# stage C psn chain offsets: 3 chains per 512-f32 PSUM bank, no chain
# crosses a bank boundary (144 <= 168; 3*168=504 <= 512)
PSN_OFF = [(n // 3) * 512 + (n % 3) * 168 for n in range(9)]


def _sg_lblock_col(g, l):
    st = g - 2
    return 16 * sum(1 for lp in range(l) if abs(st) <= lp)


def _slot_of(p1, m1, p2, m2, car, dar):
    if (p1 < p2) or (p1 == p2 and m1 <= m2):
        q1, mm1, q2, mm2, a, b = p1, m1, p2, m2, car, dar
    else:
        q1, mm1, q2, mm2, a, b = p2, m2, p1, m1, dar, car
    op = PRODUCT_OPS[_QIDX[(q1, q2, mm1)]]
    assert op["m2_lo"] <= mm2 <= op["m2_lo"] + op["nm2"] - 1
    return op["qoff"] + (mm2 - op["m2_lo"]) * 256 + a * 16 + b


def _assemble_W2(w_nl):
    W2 = np.zeros((NSLOT, 48))
    for l in range(3):
        off = 0
        for (p1, p2) in _valid_pairs(l):
            Cg = _cg_matrix(p1, p2, l)
            wl = np.asarray(w_nl[l], np.float64)
            for m1 in range(2 * p1 + 1):
                for m2 in range(2 * p2 + 1):
                    st = (m1 - p1) + (m2 - p2)
                    if abs(st) > l:
                        continue
                    gc = Cg[m1, m2, st + l]
                    if gc == 0.0:
                        continue
                    g = st + 2
                    slots = _slot_of(p1, m1, p2, m2, _CAR, _DAR)
                    t = off + _CAR * 16 + _DAR
                    c0 = _sg_lblock_col(g, l)
                    W2[slots.ravel(), c0:c0 + 16] += gc * wl[t.ravel(), :]
            off += 256
    return W2


def _assemble_W3(w_rel):
    SOFF = [0, 1, 4]
    YOFF = np.concatenate([[0], np.cumsum(SG_NCOL)])
    W3 = np.zeros((9 * 144, 144))
    ar = np.arange(16)
    for l in range(3):
        off = 0
        for (p1, p2) in _valid_pairs(l):
            Cg = _cg_matrix(p1, p2, l)
            wr = np.asarray(w_rel[l], np.float64)
            for m1 in range(2 * p1 + 1):
                for m2 in range(2 * p2 + 1):
                    st = (m1 - p1) + (m2 - p2)
                    if abs(st) > l:
                        continue
                    gc = Cg[m1, m2, st + l]
                    if gc == 0.0:
                        continue
                    gY = (m1 - p1) + 2
                    a0 = YOFF[gY] + _sg_lblock_col(gY, p1)
                    rows = (SOFF[p2] + m2) * 144 + a0 + ar
                    cols = FOFF[l] + (st + l) + ar * LDIM[l]
                    W3[np.ix_(rows, cols)] += gc * wr[off:off + 16, :]
            off += 16
    return W3


# ------------------------------------------------------------ bass builder
_NC_CACHE = {}
CPB = 4                     # PE-lane transpose chunks per PSUM copy batch


def _build_nc(debug=False):
    import concourse.bacc as bacc
    import concourse.bass as bass
    import concourse.tile as tile
    from concourse import mybir
    from concourse.masks import make_identity

    f32 = mybir.dt.float32
    bf16 = mybir.dt.bfloat16
    nc = bacc.Bacc()
    d_cvs = nc.declare_dram_parameter("cvs", [128, 281], f32, isOutput=False)
    d_w2 = nc.declare_dram_parameter("w2", [128, W2COLS], bf16, isOutput=False)
    d_w3a = nc.declare_dram_parameter("w3a", [96, 9 * 144], bf16, isOutput=False)
    d_w3b = nc.declare_dram_parameter("w3b", [32, 3 * 9 * 144], bf16, isOutput=False)
    d_zout = nc.declare_dram_parameter("zout", [128, NF * 9], bf16, isOutput=True)
    if debug:
        d_dbgx = nc.declare_dram_parameter("dbgx", [128, NF], f32, isOutput=True)
        d_dbgp = nc.declare_dram_parameter("dbgp", [128, NSLOT], bf16, isOutput=True)
        d_dbgy = nc.declare_dram_parameter("dbgy", [96, 5 * 128], bf16, isOutput=True)
        d_dbgt = nc.declare_dram_parameter("dbgt", [128, 9 * 144], bf16, isOutput=True)

    def vap(t, doff, freedims):
        base = t[:] if not isinstance(t, bass.AP) else t
        return bass.AP(tensor=base.tensor, offset=base.offset + doff,
                       ap=[list(base.ap[0])] + [list(d) for d in freedims])

    with tile.TileContext(nc) as tc:
      with (
        tc.tile_pool(name="big", bufs=1) as big,
        tc.tile_pool(name="sb", bufs=1) as sb,
        tc.tile_pool(name="pt", bufs=3) as ptp,
        tc.tile_pool(name="ps_t", bufs=3, space="PSUM") as ps_t,
        tc.tile_pool(name="ps_y", bufs=1, space="PSUM") as ps_y,
        tc.tile_pool(name="ps_c", bufs=1, space="PSUM") as ps_c,
      ):
        # ---- input DMAs (SP queue: cvs, w2; Act queue: w3 - both are HWDGE)
        cvs = sb.tile([128, 281], f32)
        nc.sync.dma_start(out=cvs, in_=d_cvs[:, :])
        connT, vcat, ssum = cvs[:, 0:128], cvs[:, 128:272], cvs[:, 272:281]
        w2 = big.tile([128, W2COLS], bf16)
        nc.sync.dma_start(out=w2, in_=d_w2[:, :])
        w3A = sb.tile([96, 9, 144], bf16)
        nc.scalar.dma_start(out=w3A, in_=d_w3a[:, :].rearrange(
            "p (n e) -> p n e", n=9))
        w3B = sb.tile([32, 3, 9, 144], bf16)
        nc.scalar.dma_start(out=w3B, in_=d_w3b[:, :].rearrange(
            "p (j n e) -> p j n e", j=3, n=9))
        ident = sb.tile([128, 128], bf16)
        make_identity(nc, ident)

        # ---- stage A: X[i, feat] = connT.T @ vcat (fp32, exact)
        # psn split: banks 0-1 (chains n=0..5) | bank 2 (chains n=6..8) so
        # the bank-2 scale causes no false WAR against bank 0-1 chains
        psnA = ps_c.tile([128, 1024], f32, tag="psna")
        psnB = ps_c.tile([128, 512], f32, tag="psnb")
        x_ps = psnA[:, 0:NF]
        nc.tensor.matmul(x_ps, connT, vcat, start=True, stop=True)
        X = sb.tile([128, NF], f32)
        nc.vector.tensor_copy(out=X, in_=x_ps)

        # ---- PSUM pre-zero via dummy start=True matmuls (whole 2KB bank)
        # ymix is a-major: group g occupies rows YOFF[g]:YOFF[g]+ncol of one
        # [128,128] column window (g=4 spills to a [16,128] side tile)
        ymixA = ps_y.tile([128, 128], f32)
        ymixBC = ps_y.tile([32, 3, 128], f32)
        # rhs = ident[0:1, 1:2] is 0.0, so the dummy writes zeros
        nc.tensor.matmul(ymixA[:, 0:1], ident[0:1, 0:128], ident[0:1, 1:2],
                         start=True, stop=True, skip_group_check=True)
        nc.tensor.matmul(ymixBC[0:32, 0, 0:1], ident[0:1, 0:32],
                         ident[0:1, 1:2],
                         start=True, stop=True, skip_group_check=True)
        for bank in range(2):
            nc.tensor.matmul(psnA[:, bank * 512:bank * 512 + 1],
                             ident[0:1, 0:128], ident[0:1, 1:2],
                             start=True, stop=True, skip_group_check=True)
        nc.tensor.matmul(psnB[:, 0:1], ident[0:1, 0:128], ident[0:1, 1:2],
                         start=True, stop=True, skip_group_check=True)

        # ---- stage B products + transposes + mix matmuls, then per-g C
        P = big.tile([128, NSLOT], bf16)
        ysb = sb.tile([96, 5, 128], bf16)
        # T2[i, (b, k, e)] bf16; two DVE-only scale ops (banks 0-1 fused 3D,
        # then bank 2) - single engine, no cross-engine sems, one DMA out
        T2 = sb.tile([128, 3, 3, NF], bf16)

        def scale_part1():
            nc.vector.tensor_tensor(
                out=vap(T2, 0, [[3 * NF, 2], [NF, 3], [1, NF]]),
                in0=vap(psnA, 0, [[512, 2], [168, 3], [1, NF]]),
                in1=vap(ssum, 0, [[3, 2], [1, 3], [0, NF]]),
                op=mybir.AluOpType.mult)
            nc.sync.dma_start(out=d_zout[:, 0:6 * NF],
                              in_=T2[:, 0:2, :, :])

        def scale_part2():
            nc.vector.tensor_tensor(
                out=vap(T2, 2 * 3 * NF, [[NF, 3], [1, NF]]),
                in0=vap(psnB, 0, [[168, 3], [1, NF]]),
                in1=vap(ssum, 6, [[1, 3], [0, NF]]),
                op=mybir.AluOpType.mult)
            nc.sync.dma_start(out=d_zout[:, 6 * NF:9 * NF],
                              in_=T2[:, 2, :, :])
        g_seen = [0] * 5
        g_total = [0] * 5
        for g in CHUNK_G:
            g_total[g] += 1

        # per-op chunk list
        chunk_of_op = {}
        ci = 0
        for op in PRODUCT_OPS:
            nch = op["nm2"] * 2
            chunk_of_op[(op["l1"], op["l2"], op["m1"])] = \
                list(range(ci, ci + nch))
            ci += nch

        # PSUM out base partition is limited to {0,32,64} (and {0,64} for
        # >32 cols): ymixA holds g2 rows 0:48 and g1 rows 64:96 (gaps stay
        # zero from the bank pre-zero); ymixBC windows hold g3, g4, g0
        def mix_matmul(ci, rhs_ap):
            g = CHUNK_G[ci]
            ncol = SG_NCOL[g]
            if g == 2:
                dst = ymixA[0:48, :]
            elif g == 1:
                dst = ymixA[64:96, :]
            elif g == 3:
                dst = ymixBC[0:32, 0, :]
            else:
                dst = ymixBC[0:16, 1 if g == 4 else 2, :]
            nc.tensor.matmul(
                dst, w2[:, W2COFF[ci]:W2COFF[ci] + ncol], rhs_ap,
                start=False, stop=True, skip_group_check=True)

        pe_batch = []                                 # pending PE-lane chunks
        n_batch = [0]
        dma_ready = []           # (key, ptd) whose matmuls are still pending

        def flush_pe_batch():
            nonlocal pe_batch
            if not pe_batch:
                return
            t_ps = ps_t.tile([128, CPB * 128], bf16)
            for j, ci in enumerate(pe_batch):
                nc.tensor.transpose(
                    t_ps[:, j * 128:(j + 1) * 128],
                    P[:, ci * 128:(ci + 1) * 128], ident)
            pt = ptp.tile([128, CPB * 128], bf16)
            nb = len(pe_batch)
            n_batch[0] += 1
            if n_batch[0] <= 8:       # DVE is busy with products early on
                nc.scalar.activation(pt[:, 0:nb * 128], t_ps[:, 0:nb * 128],
                                     mybir.ActivationFunctionType.Copy)
            else:                     # late: split halves across DVE + Act
                h = (nb + 1) // 2 * 128
                nc.vector.tensor_copy(out=pt[:, 0:h], in_=t_ps[:, 0:h])
                if nb * 128 > h:
                    nc.scalar.activation(pt[:, h:nb * 128], t_ps[:, h:nb * 128],
                                         mybir.ActivationFunctionType.Copy)
            for j, ci in enumerate(pe_batch):
                mix_matmul(ci, pt[:, j * 128:(j + 1) * 128])
            pe_batch = []
            # weave one ready XBAR block's matmuls in after each batch
            # from batch 2 (matches the ~1us XBAR drain cadence)
            if n_batch[0] >= 2 and dma_ready:
                chunks, ptd = dma_ready.pop(0)
                for j, ci in enumerate(chunks):
                    mix_matmul(ci, ptd[:, j, :])

        # phase 1a: ALL products first - each engine's stream is then pure
        # (DVE: products then late copies; Pool: products; Act: copies).
        # Cross-engine overlap is automatic via semaphores.
        for key in _OP_ORDER:
            op = PRODUCT_OPS[_QIDX[key]]
            l1, l2, m1 = key
            nm2 = op["nm2"]
            peng = nc.gpsimd if key in _POOL_OPS else nc.vector
            peng.tensor_tensor(
                out=vap(P, op["qoff"], [[256, nm2], [16, 16], [1, 16]]),
                in0=vap(X, FOFF[l1] + m1, [[0, nm2], [LDIM[l1], 16], [0, 16]]),
                in1=vap(X, FOFF[l2] + op["m2_lo"],
                        [[1, nm2], [0, 16], [LDIM[l2], 16]]),
                op=mybir.AluOpType.mult)
        # phase 1b: XBAR transposes (SP queue dispatches in product order)
        for grp in _XBAR_GROUPS:
            chunks = sorted(ci for k in grp for ci in chunk_of_op[k])
            assert chunks == list(range(chunks[0], chunks[0] + len(chunks)))
            nch = len(chunks)
            ptd = ptp.tile([128, nch, 128], bf16, tag=f"dma{grp[0]}")
            nc.sync.dma_start(
                out=ptd,
                in_=P[:, chunks[0] * 128:(chunks[-1] + 1) * 128],
                transpose=True)
            dma_ready.append((chunks, ptd))
        # phase 1c: PE-lane batch pipeline, XBAR matmul blocks woven in
        for key in _OP_ORDER:
            if key in _DMA_T_OPS:
                continue
            for ci in chunk_of_op[key]:
                pe_batch.append(ci)
                if len(pe_batch) == CPB:
                    flush_pe_batch()
        flush_pe_batch()

        # any DMA-lane matmuls not yet woven in
        for chunks, ptd in dma_ready:
            for j, ci in enumerate(chunks):
                mix_matmul(ci, ptd[:, j, :])

        # stage C: one ysb copy per tile, then 2 accumulating matmuls per
        # chain (K=128 + K=16)
        ysbA = sb.tile([96, 128], bf16)
        ysbBC = sb.tile([32, 3, 128], bf16)
        nc.scalar.activation(ysbA, ymixA[0:96, :],
                             mybir.ActivationFunctionType.Copy)
        nc.vector.tensor_copy(out=ysbBC, in_=ymixBC)
        def chain(n):
            ps, off = (psnA, PSN_OFF[n]) if n < 6 else (psnB, PSN_OFF[n] - 1024)
            dst = ps[:, off:off + NF]
            nc.tensor.matmul(dst, ysbA, w3A[0:96, n, :],
                             start=False, stop=True, skip_group_check=True)
            nc.tensor.matmul(dst, ysbBC[0:32, 0, :], w3B[0:32, 0, n, :],
                             start=False, stop=True, skip_group_check=True)
            for j in (1, 2):
                nc.tensor.matmul(dst, ysbBC[0:16, j, :], w3B[0:16, j, n, :],
                                 start=False, stop=True, skip_group_check=True)

        for n in range(6):
            chain(n)
        scale_part1()          # big scale + zout slice overlap bank-2 chains
        for n in (6, 7, 8):
            chain(n)
        scale_part2()

        if debug:
            nc.sync.dma_start(out=d_dbgx[:, :], in_=X)
            for q in range(4):
                s, e = q * 2496, (q + 1) * 2496
                nc.sync.dma_start(out=d_dbgp[:, s:e], in_=P[:, s:e])



    nc.compile()
    return nc

# ------------------------------------------------------------- host entry
def _get_nc():
    if "nc" not in _NC_CACHE:
        _NC_CACHE["nc"] = _build_nc()
    return _NC_CACHE["nc"]


def kernel(vertices_0, vertices_1, vertices_2, connectivity,
           sph_0, sph_1, sph_2,
           w_nl_0, w_nl_1, w_nl_2,
           w_rel_0, w_rel_1, w_rel_2):
    import ml_dtypes
    from concourse.bass_utils import run_bass_kernel_spmd

    f = np.float32
    bf = ml_dtypes.bfloat16
    verts = [np.asarray(v, f) for v in (vertices_0, vertices_1, vertices_2)]
    sphs = [np.asarray(s, f) for s in (sph_0, sph_1, sph_2)]
    conn = np.asarray(connectivity)

    W2 = _assemble_W2([np.asarray(w, f) for w in (w_nl_0, w_nl_1, w_nl_2)])
    w2p = np.zeros((128, W2COLS), dtype=bf)
    for ci in range(NCHUNK):
        ncol = SG_NCOL[CHUNK_G[ci]]
        w2p[:, W2COFF[ci]:W2COFF[ci] + ncol] = \
            W2[ci * 128:(ci + 1) * 128, 0:ncol].astype(bf)

    W3 = _assemble_W3([np.asarray(w, f) for w in (w_rel_0, w_rel_1, w_rel_2)])
    W3r = W3.reshape(9, 144, 144)       # a-index: YOFF=[0,16,48,96,128]
    w3a = np.zeros((96, 9, 144), np.float64)
    w3a[0:48] = W3r[:, 48:96, :].transpose(1, 0, 2)     # g2
    w3a[64:96] = W3r[:, 16:48, :].transpose(1, 0, 2)    # g1
    w3b = np.zeros((32, 3, 9, 144), np.float64)
    w3b[0:32, 0] = W3r[:, 96:128, :].transpose(1, 0, 2)   # g3
    w3b[0:16, 1] = W3r[:, 128:144, :].transpose(1, 0, 2)  # g4
    w3b[0:16, 2] = W3r[:, 0:16, :].transpose(1, 0, 2)     # g0
    w3a = np.ascontiguousarray(w3a.reshape(96, -1)).astype(bf)
    w3b = np.ascontiguousarray(w3b.reshape(32, -1)).astype(bf)

    in_maps = []
    for b in range(NB):
        connT = conn[b].astype(f).T
        vcat = np.concatenate([v[b].reshape(128, -1) for v in verts], axis=1)
        ssum = np.concatenate([s[b].sum(axis=1).reshape(128, -1) for s in sphs],
                              axis=1)
        cvs = np.ascontiguousarray(
            np.concatenate([connT, vcat, ssum], axis=1).astype(f))
        in_maps.append(dict(cvs=cvs, w2=w2p, w3a=w3a, w3b=w3b))

    res = run_bass_kernel_spmd(_get_nc(), in_maps, list(range(NB)))
    _NC_CACHE["last_results"] = res
    # zout is [i, (bank, k, e)] bf16; sum over (bank, k) on host
    Z = np.stack([res.results[b]["zout"].astype(f)
                  .reshape(128, 9, NF).sum(axis=1) for b in range(NB)])

    out = np.zeros((NB, 128, 1, 16, 9), dtype=f)
    koff = [0, 1, 4]
    for l in range(3):
        blk = Z[:, :, FOFF[l]:FOFF[l] + 16 * LDIM[l]]
        blk = blk.reshape(NB, 128, 16, LDIM[l])
        nf = np.sum(blk.astype(np.float64) ** 2)
        out[:, :, 0, :, koff[l]:koff[l] + LDIM[l]] = blk / np.sqrt(nf / 16.0)
    return out


# revision 97
# speedup vs baseline: 1.0006x; 1.0006x over previous
"""Trainium2 Bass kernel for nn_CGLayer (gnn_message_passing).

Contract: kernel(**inputs) takes FULL inputs (as reference.setup_inputs()),
returns FULL output [8,128,1,16,9] f32. Internally: data-parallel over the
batch dim across 8 NeuronCores; per core one batch element.

Algebraic reduction (exact):
  X   = conn @ vertices                  (message passing, per batch)
  Y   = mix_nl(cg(X, X))                 (per-node quadratic in X)
  S   = sum_j sph[:, j, :]               (host-computed; neighbor sum
  Z   = mix_rel(cg(Y, S))                 commutes through the rel-CG)
  out = Z / sqrt(sum Z^2 / 16)           (host epilogue, global)

Device pipeline per core (bf16 hot path, ~23us TimelineSim vs 134us for
the fp32 feature-partition baseline):
  A:  X[i,144] f32   = matmul(lhsT=connT, rhs=vcat)
  B:  P[i,9984] bf16 = 13 DVE/Pool stride-tricked pair-product ops over a
                       dense symmetry-folded slot layout (m1<=m2 for l1=l2).
      Chunk transposes to slot-partition layout run on two lanes:
       - 54 chunks via XBAR dma_start_transpose (7 instrs, SBUF->SBUF,
         no PE / no PSUM copies; ops contiguous in slot space share one
         instruction)
       - 24 chunks via PE transpose -> PSUM -> batched Act copies
      78 PE bf16 mix matmuls accumulate (start=False) into bank-pre-zeroed
      ymix tiles (a-major rows; PSUM zero-region = whole 2KB bank, so
      dummy start=True matmuls pre-zero and groups never share wipes)
  C:  one ysbA/ysbBC copy pair, then 9 chains x 4 accumulating matmuls
      (K=96/32/16/16) into psnA/psnB; two 3D DVE scale ops multiply by
      ssum[:, n] into T2 bf16; two DMAs ship T2 out
Host: n-sum of T2, unpack e=(l,c',k), global per-l normalization.
"""
import numpy as np
from math import factorial, sqrt

MAXL = 2
CH = 16
NN = 128
NB = 8
LDIM = [1, 3, 5]
FOFF = [0, 16, 64]
NF = 144
SG_NCOL = [16, 32, 48, 32, 16]

# ------------------------------------------------------------- CG tables
def _cg_coeff(j1, m1, j2, m2, j3, m3):
    if m3 != m1 + m2:
        return 0.0
    pre = sqrt((2 * j3 + 1) * factorial(j3 + j1 - j2) * factorial(j3 - j1 + j2)
               * factorial(j1 + j2 - j3) / factorial(j1 + j2 + j3 + 1))
    pre *= sqrt(factorial(j3 + m3) * factorial(j3 - m3) * factorial(j1 - m1)
                * factorial(j1 + m1) * factorial(j2 - m2) * factorial(j2 + m2))
    s = 0.0
    vmin = max(0, j2 - j3 - m1, j1 - j3 + m2)
    vmax = min(j1 + j2 - j3, j1 - m1, j2 + m2)
    for v in range(vmin, vmax + 1):
        s += (-1) ** v / (factorial(v) * factorial(j1 + j2 - j3 - v)
                          * factorial(j1 - m1 - v) * factorial(j2 + m2 - v)
                          * factorial(j3 - j2 + m1 + v) * factorial(j3 - j1 - m2 + v))
    return pre * s


def _cg_matrix(l1, l2, l):
    M = np.zeros((2 * l1 + 1, 2 * l2 + 1, 2 * l + 1))
    for m1 in range(-l1, l1 + 1):
        for m2 in range(-l2, l2 + 1):
            if -l <= m1 + m2 <= l:
                M[m1 + l1, m2 + l2, m1 + m2 + l] = _cg_coeff(l1, m1, l2, m2, l, m1 + m2)
    return M


def _valid_pairs(l):
    return [(l1, l2) for l1 in range(3) for l2 in range(3)
            if abs(l1 - l2) <= l <= l1 + l2]

# ---------------------------------------------- dense symmetric slot layout
def _product_ops():
    ops = []
    qoff = 0
    for l1 in range(3):
        for l2 in range(l1, 3):
            for m1 in range(2 * l1 + 1):
                mt1 = m1 - l1
                m2_lo = max(0, l2 - 2 - mt1)
                m2_hi = min(2 * l2, l2 + 2 - mt1)
                if l1 == l2:
                    m2_lo = max(m2_lo, m1)
                nm2 = m2_hi - m2_lo + 1
                if nm2 <= 0:
                    continue
                ops.append(dict(l1=l1, l2=l2, m1=m1, m2_lo=m2_lo, nm2=nm2,
                                qoff=qoff))
                qoff += nm2 * 256
    return ops, qoff

PRODUCT_OPS, NSLOT = _product_ops()          # 13 ops, 9984 slots
NCHUNK = NSLOT // 128                        # 78


def _chunk_groups():
    gs = []
    for op in PRODUCT_OPS:
        mt1 = op["m1"] - op["l1"]
        for m2i in range(op["nm2"]):
            g = mt1 + (op["m2_lo"] + m2i - op["l2"]) + 2
            gs.extend([g, g])
    return gs

CHUNK_G = _chunk_groups()
assert len(CHUNK_G) == NCHUNK
W2COFF = np.concatenate([[0], np.cumsum([SG_NCOL[g] for g in CHUNK_G])])
W2COLS = int(W2COFF[-1])                     # 2464

_QIDX = {(op["l1"], op["l2"], op["m1"]): i for i, op in enumerate(PRODUCT_OPS)}
_CAR, _DAR = np.meshgrid(np.arange(16), np.arange(16), indexing="ij")

# engine/lane assignment per product op (key = (l1,l2,m1)):
#   products: Pool ~3.3k of 9984 elems (2.06 ns/elem), DVE the rest (1.09)
#   transposes: XBAR DMA lane for 48 chunks, PE lane for 30
_POOL_OPS = {(0, 2, 0), (2, 2, 0), (2, 2, 2), (2, 2, 3)}
# XBAR transpose instruction groups (ops contiguous in slot space can share
# one instruction); everything else (Pool's 4 ops, 24 chunks) goes PE-lane
_XBAR_GROUPS = [[(1, 2, 0)], [(1, 2, 1)],
                [(1, 1, 0), (1, 1, 1), (1, 1, 2)], [(1, 2, 2)],
                [(2, 2, 1)], [(0, 1, 0)], [(0, 0, 0)]]
_DMA_T_OPS = {k for grp in _XBAR_GROUPS for k in grp}
# issue order: XBAR-feeding DVE ops early, ordered so XBAR groups complete
# products in dispatch order; Pool ops interleave
_OP_ORDER = [(1, 2, 0), (0, 2, 0), (1, 2, 1), (2, 2, 0), (1, 1, 0),
             (1, 1, 1), (1, 1, 2), (2, 2, 2), (1, 2, 2), (2, 2, 3),
             (2, 2, 1), (0, 1, 0), (0, 0, 0)]

# stage C psn chain offsets: 3 chains per 512-f32 PSUM bank, no chain
# crosses a bank boundary (144 <= 168; 3*168=504 <= 512)
PSN_OFF = [(n // 3) * 512 + (n % 3) * 168 for n in range(9)]


def _sg_lblock_col(g, l):
    st = g - 2
    return 16 * sum(1 for lp in range(l) if abs(st) <= lp)


def _slot_of(p1, m1, p2, m2, car, dar):
    if (p1 < p2) or (p1 == p2 and m1 <= m2):
        q1, mm1, q2, mm2, a, b = p1, m1, p2, m2, car, dar
    else:
        q1, mm1, q2, mm2, a, b = p2, m2, p1, m1, dar, car
    op = PRODUCT_OPS[_QIDX[(q1, q2, mm1)]]
    assert op["m2_lo"] <= mm2 <= op["m2_lo"] + op["nm2"] - 1
    return op["qoff"] + (mm2 - op["m2_lo"]) * 256 + a * 16 + b


def _assemble_W2(w_nl):
    W2 = np.zeros((NSLOT, 48))
    for l in range(3):
        off = 0
        for (p1, p2) in _valid_pairs(l):
            Cg = _cg_matrix(p1, p2, l)
            wl = np.asarray(w_nl[l], np.float64)
            for m1 in range(2 * p1 + 1):
                for m2 in range(2 * p2 + 1):
                    st = (m1 - p1) + (m2 - p2)
                    if abs(st) > l:
                        continue
                    gc = Cg[m1, m2, st + l]
                    if gc == 0.0:
                        continue
                    g = st + 2
                    slots = _slot_of(p1, m1, p2, m2, _CAR, _DAR)
                    t = off + _CAR * 16 + _DAR
                    c0 = _sg_lblock_col(g, l)
                    W2[slots.ravel(), c0:c0 + 16] += gc * wl[t.ravel(), :]
            off += 256
    return W2


def _assemble_W3(w_rel):
    SOFF = [0, 1, 4]
    YOFF = np.concatenate([[0], np.cumsum(SG_NCOL)])
    W3 = np.zeros((9 * 144, 144))
    ar = np.arange(16)
    for l in range(3):
        off = 0
        for (p1, p2) in _valid_pairs(l):
            Cg = _cg_matrix(p1, p2, l)
            wr = np.asarray(w_rel[l], np.float64)
            for m1 in range(2 * p1 + 1):
                for m2 in range(2 * p2 + 1):
                    st = (m1 - p1) + (m2 - p2)
                    if abs(st) > l:
                        continue
                    gc = Cg[m1, m2, st + l]
                    if gc == 0.0:
                        continue
                    gY = (m1 - p1) + 2
                    a0 = YOFF[gY] + _sg_lblock_col(gY, p1)
                    rows = (SOFF[p2] + m2) * 144 + a0 + ar
                    cols = FOFF[l] + (st + l) + ar * LDIM[l]
                    W3[np.ix_(rows, cols)] += gc * wr[off:off + 16, :]
            off += 16
    return W3


# ------------------------------------------------------------ bass builder
_NC_CACHE = {}
CPB = 4                     # PE-lane transpose chunks per PSUM copy batch


def _build_nc(debug=False):
    import concourse.bacc as bacc
    import concourse.bass as bass
    import concourse.tile as tile
    from concourse import mybir
    from concourse.masks import make_identity

    f32 = mybir.dt.float32
    bf16 = mybir.dt.bfloat16
    nc = bacc.Bacc()
    d_cvs = nc.declare_dram_parameter("cvs", [128, 281], f32, isOutput=False)
    d_w2 = nc.declare_dram_parameter("w2", [128, W2COLS], bf16, isOutput=False)
    d_w3a = nc.declare_dram_parameter("w3a", [96, 9 * 144], bf16, isOutput=False)
    d_w3b = nc.declare_dram_parameter("w3b", [32, 3 * 9 * 144], bf16, isOutput=False)
    d_zout = nc.declare_dram_parameter("zout", [128, NF * 9], bf16, isOutput=True)
    if debug:
        d_dbgx = nc.declare_dram_parameter("dbgx", [128, NF], f32, isOutput=True)
        d_dbgp = nc.declare_dram_parameter("dbgp", [128, NSLOT], bf16, isOutput=True)
        d_dbgy = nc.declare_dram_parameter("dbgy", [96, 5 * 128], bf16, isOutput=True)
        d_dbgt = nc.declare_dram_parameter("dbgt", [128, 9 * 144], bf16, isOutput=True)

    def vap(t, doff, freedims):
        base = t[:] if not isinstance(t, bass.AP) else t
        return bass.AP(tensor=base.tensor, offset=base.offset + doff,
                       ap=[list(base.ap[0])] + [list(d) for d in freedims])

    with tile.TileContext(nc) as tc:
      with (
        tc.tile_pool(name="big", bufs=1) as big,
        tc.tile_pool(name="sb", bufs=1) as sb,
        tc.tile_pool(name="pt", bufs=3) as ptp,
        tc.tile_pool(name="ps_t", bufs=3, space="PSUM") as ps_t,
        tc.tile_pool(name="ps_y", bufs=1, space="PSUM") as ps_y,
        tc.tile_pool(name="ps_c", bufs=1, space="PSUM") as ps_c,
      ):
        # ---- input DMAs (SP queue: cvs, w2; Act queue: w3 - both are HWDGE)
        cvs = sb.tile([128, 281], f32)
        nc.sync.dma_start(out=cvs, in_=d_cvs[:, :])
        connT, vcat, ssum = cvs[:, 0:128], cvs[:, 128:272], cvs[:, 272:281]
        w2 = big.tile([128, W2COLS], bf16)
        nc.sync.dma_start(out=w2, in_=d_w2[:, :])
        w3A = sb.tile([96, 9, 144], bf16)
        nc.scalar.dma_start(out=w3A, in_=d_w3a[:, :].rearrange(
            "p (n e) -> p n e", n=9))
        w3B = sb.tile([32, 3, 9, 144], bf16)
        nc.scalar.dma_start(out=w3B, in_=d_w3b[:, :].rearrange(
            "p (j n e) -> p j n e", j=3, n=9))
        ident = sb.tile([128, 128], bf16)
        make_identity(nc, ident)

        # ---- stage A: X[i, feat] = connT.T @ vcat (fp32, exact)
        # psn split: banks 0-1 (chains n=0..5) | bank 2 (chains n=6..8) so
        # the bank-2 scale causes no false WAR against bank 0-1 chains
        psnA = ps_c.tile([128, 1024], f32, tag="psna")
        psnB = ps_c.tile([128, 512], f32, tag="psnb")
        x_ps = psnA[:, 0:NF]
        nc.tensor.matmul(x_ps, connT, vcat, start=True, stop=True)
        X = sb.tile([128, NF], f32)
        nc.vector.tensor_copy(out=X, in_=x_ps)

        # ---- PSUM pre-zero via dummy start=True matmuls (whole 2KB bank)
        # ymix is a-major: group g occupies rows YOFF[g]:YOFF[g]+ncol of one
        # [128,128] column window (g=4 spills to a [16,128] side tile)
        ymixA = ps_y.tile([128, 128], f32)
        ymixBC = ps_y.tile([32, 3, 128], f32)
        # rhs = ident[0:1, 1:2] is 0.0, so the dummy writes zeros
        nc.tensor.matmul(ymixA[:, 0:1], ident[0:1, 0:128], ident[0:1, 1:2],
                         start=True, stop=True, skip_group_check=True)
        nc.tensor.matmul(ymixBC[0:32, 0, 0:1], ident[0:1, 0:32],
                         ident[0:1, 1:2],
                         start=True, stop=True, skip_group_check=True)
        for bank in range(2):
            nc.tensor.matmul(psnA[:, bank * 512:bank * 512 + 1],
                             ident[0:1, 0:128], ident[0:1, 1:2],
                             start=True, stop=True, skip_group_check=True)
        nc.tensor.matmul(psnB[:, 0:1], ident[0:1, 0:128], ident[0:1, 1:2],
                         start=True, stop=True, skip_group_check=True)

        # ---- stage B products + transposes + mix matmuls, then per-g C
        P = big.tile([128, NSLOT], bf16)
        ysb = sb.tile([96, 5, 128], bf16)
        # T2[i, (b, k, e)] bf16; two DVE-only scale ops (banks 0-1 fused 3D,
        # then bank 2) - single engine, no cross-engine sems, one DMA out
        T2 = sb.tile([128, 3, 3, NF], bf16)

        def scale_part1():
            nc.vector.tensor_tensor(
                out=vap(T2, 0, [[3 * NF, 2], [NF, 3], [1, NF]]),
                in0=vap(psnA, 0, [[512, 2], [168, 3], [1, NF]]),
                in1=vap(ssum, 0, [[3, 2], [1, 3], [0, NF]]),
                op=mybir.AluOpType.mult)
            nc.sync.dma_start(out=d_zout[:, 0:6 * NF],
                              in_=T2[:, 0:2, :, :])

        def scale_part2():
            nc.vector.tensor_tensor(
                out=vap(T2, 2 * 3 * NF, [[NF, 3], [1, NF]]),
                in0=vap(psnB, 0, [[168, 3], [1, NF]]),
                in1=vap(ssum, 6, [[1, 3], [0, NF]]),
                op=mybir.AluOpType.mult)
            nc.sync.dma_start(out=d_zout[:, 6 * NF:9 * NF],
                              in_=T2[:, 2, :, :])
        g_seen = [0] * 5
        g_total = [0] * 5
        for g in CHUNK_G:
            g_total[g] += 1

        # per-op chunk list
        chunk_of_op = {}
        ci = 0
        for op in PRODUCT_OPS:
            nch = op["nm2"] * 2
            chunk_of_op[(op["l1"], op["l2"], op["m1"])] = \
                list(range(ci, ci + nch))
            ci += nch

        # PSUM out base partition is limited to {0,32,64} (and {0,64} for
        # >32 cols): ymixA holds g2 rows 0:48 and g1 rows 64:96 (gaps stay
        # zero from the bank pre-zero); ymixBC windows hold g3, g4, g0
        def mix_matmul(ci, rhs_ap):
            g = CHUNK_G[ci]
            ncol = SG_NCOL[g]
            if g == 2:
                dst = ymixA[0:48, :]
            elif g == 1:
                dst = ymixA[64:96, :]
            elif g == 3:
                dst = ymixBC[0:32, 0, :]
            else:
                dst = ymixBC[0:16, 1 if g == 4 else 2, :]
            nc.tensor.matmul(
                dst, w2[:, W2COFF[ci]:W2COFF[ci] + ncol], rhs_ap,
                start=False, stop=True, skip_group_check=True)

        pe_batch = []                                 # pending PE-lane chunks
        n_batch = [0]
        dma_ready = []           # (key, ptd) whose matmuls are still pending

        def flush_pe_batch():
            nonlocal pe_batch
            if not pe_batch:
                return
            t_ps = ps_t.tile([128, CPB * 128], bf16)
            for j, ci in enumerate(pe_batch):
                nc.tensor.transpose(
                    t_ps[:, j * 128:(j + 1) * 128],
                    P[:, ci * 128:(ci + 1) * 128], ident)
            pt = ptp.tile([128, CPB * 128], bf16)
            nb = len(pe_batch)
            n_batch[0] += 1
            if n_batch[0] <= 8:       # DVE is busy with products early on
                nc.scalar.activation(pt[:, 0:nb * 128], t_ps[:, 0:nb * 128],
                                     mybir.ActivationFunctionType.Copy)
            else:                     # late: split halves across DVE + Act
                h = (nb + 1) // 2 * 128
                nc.vector.tensor_copy(out=pt[:, 0:h], in_=t_ps[:, 0:h])
                if nb * 128 > h:
                    nc.scalar.activation(pt[:, h:nb * 128], t_ps[:, h:nb * 128],
                                         mybir.ActivationFunctionType.Copy)
            for j, ci in enumerate(pe_batch):
                mix_matmul(ci, pt[:, j * 128:(j + 1) * 128])
            pe_batch = []
            # weave one ready XBAR block's matmuls in after each batch
            # from batch 2 (matches the ~1us XBAR drain cadence)
            if n_batch[0] >= 2 and dma_ready:
                chunks, ptd = dma_ready.pop(0)
                for j, ci in enumerate(chunks):
                    mix_matmul(ci, ptd[:, j, :])

        # phase 1a: ALL products first - each engine's stream is then pure
        # (DVE: products then late copies; Pool: products; Act: copies).
        # Cross-engine overlap is automatic via semaphores.
        for key in _OP_ORDER:
            op = PRODUCT_OPS[_QIDX[key]]
            l1, l2, m1 = key
            nm2 = op["nm2"]
            peng = nc.gpsimd if key in _POOL_OPS else nc.vector
            peng.tensor_tensor(
                out=vap(P, op["qoff"], [[256, nm2], [16, 16], [1, 16]]),
                in0=vap(X, FOFF[l1] + m1, [[0, nm2], [LDIM[l1], 16], [0, 16]]),
                in1=vap(X, FOFF[l2] + op["m2_lo"],
                        [[1, nm2], [0, 16], [LDIM[l2], 16]]),
                op=mybir.AluOpType.mult)
        # phase 1b: XBAR transposes (SP queue dispatches in product order)
        for grp in _XBAR_GROUPS:
            chunks = sorted(ci for k in grp for ci in chunk_of_op[k])
            assert chunks == list(range(chunks[0], chunks[0] + len(chunks)))
            nch = len(chunks)
            ptd = ptp.tile([128, nch, 128], bf16, tag=f"dma{grp[0]}")
            nc.sync.dma_start(
                out=ptd,
                in_=P[:, chunks[0] * 128:(chunks[-1] + 1) * 128],
                transpose=True)
            dma_ready.append((chunks, ptd))
        # phase 1c: PE-lane batch pipeline, XBAR matmul blocks woven in
        for key in _OP_ORDER:
            if key in _DMA_T_OPS:
                continue
            for ci in chunk_of_op[key]:
                pe_batch.append(ci)
                if len(pe_batch) == CPB:
                    flush_pe_batch()
        flush_pe_batch()

        # any DMA-lane matmuls not yet woven in
        for chunks, ptd in dma_ready:
            for j, ci in enumerate(chunks):
                mix_matmul(ci, ptd[:, j, :])

        # stage C: one ysb copy per tile, then 2 accumulating matmuls per
        # chain (K=128 + K=16)
        ysbA = sb.tile([96, 128], bf16)
        ysbBC = sb.tile([32, 3, 128], bf16)
        nc.vector.tensor_copy(out=ysbA, in_=ymixA[0:96, :])
        nc.scalar.activation(ysbBC, ymixBC,
                             mybir.ActivationFunctionType.Copy)
        def chain(n):
            ps, off = (psnA, PSN_OFF[n]) if n < 6 else (psnB, PSN_OFF[n] - 1024)
            dst = ps[:, off:off + NF]
            nc.tensor.matmul(dst, ysbA, w3A[0:96, n, :],
                             start=False, stop=True, skip_group_check=True)
            nc.tensor.matmul(dst, ysbBC[0:32, 0, :], w3B[0:32, 0, n, :],
                             start=False, stop=True, skip_group_check=True)
            for j in (1, 2):
                nc.tensor.matmul(dst, ysbBC[0:16, j, :], w3B[0:16, j, n, :],
                                 start=False, stop=True, skip_group_check=True)

        for n in range(6):
            chain(n)
        scale_part1()          # big scale + zout slice overlap bank-2 chains
        for n in (6, 7, 8):
            chain(n)
        scale_part2()

        if debug:
            nc.sync.dma_start(out=d_dbgx[:, :], in_=X)
            for q in range(4):
                s, e = q * 2496, (q + 1) * 2496
                nc.sync.dma_start(out=d_dbgp[:, s:e], in_=P[:, s:e])



    nc.compile()
    return nc

# ------------------------------------------------------------- host entry
def _get_nc():
    if "nc" not in _NC_CACHE:
        _NC_CACHE["nc"] = _build_nc()
    return _NC_CACHE["nc"]


def kernel(vertices_0, vertices_1, vertices_2, connectivity,
           sph_0, sph_1, sph_2,
           w_nl_0, w_nl_1, w_nl_2,
           w_rel_0, w_rel_1, w_rel_2):
    import ml_dtypes
    from concourse.bass_utils import run_bass_kernel_spmd

    f = np.float32
    bf = ml_dtypes.bfloat16
    verts = [np.asarray(v, f) for v in (vertices_0, vertices_1, vertices_2)]
    sphs = [np.asarray(s, f) for s in (sph_0, sph_1, sph_2)]
    conn = np.asarray(connectivity)

    W2 = _assemble_W2([np.asarray(w, f) for w in (w_nl_0, w_nl_1, w_nl_2)])
    w2p = np.zeros((128, W2COLS), dtype=bf)
    for ci in range(NCHUNK):
        ncol = SG_NCOL[CHUNK_G[ci]]
        w2p[:, W2COFF[ci]:W2COFF[ci] + ncol] = \
            W2[ci * 128:(ci + 1) * 128, 0:ncol].astype(bf)

    W3 = _assemble_W3([np.asarray(w, f) for w in (w_rel_0, w_rel_1, w_rel_2)])
    W3r = W3.reshape(9, 144, 144)       # a-index: YOFF=[0,16,48,96,128]
    w3a = np.zeros((96, 9, 144), np.float64)
    w3a[0:48] = W3r[:, 48:96, :].transpose(1, 0, 2)     # g2
    w3a[64:96] = W3r[:, 16:48, :].transpose(1, 0, 2)    # g1
    w3b = np.zeros((32, 3, 9, 144), np.float64)
    w3b[0:32, 0] = W3r[:, 96:128, :].transpose(1, 0, 2)   # g3
    w3b[0:16, 1] = W3r[:, 128:144, :].transpose(1, 0, 2)  # g4
    w3b[0:16, 2] = W3r[:, 0:16, :].transpose(1, 0, 2)     # g0
    w3a = np.ascontiguousarray(w3a.reshape(96, -1)).astype(bf)
    w3b = np.ascontiguousarray(w3b.reshape(32, -1)).astype(bf)

    in_maps = []
    for b in range(NB):
        connT = conn[b].astype(f).T
        vcat = np.concatenate([v[b].reshape(128, -1) for v in verts], axis=1)
        ssum = np.concatenate([s[b].sum(axis=1).reshape(128, -1) for s in sphs],
                              axis=1)
        cvs = np.ascontiguousarray(
            np.concatenate([connT, vcat, ssum], axis=1).astype(f))
        in_maps.append(dict(cvs=cvs, w2=w2p, w3a=w3a, w3b=w3b))

    res = run_bass_kernel_spmd(_get_nc(), in_maps, list(range(NB)))
    _NC_CACHE["last_results"] = res
    # zout is [i, (bank, k, e)] bf16; sum over (bank, k) on host
    Z = np.stack([res.results[b]["zout"].astype(f)
                  .reshape(128, 9, NF).sum(axis=1) for b in range(NB)])

    out = np.zeros((NB, 128, 1, 16, 9), dtype=f)
    koff = [0, 1, 4]
    for l in range(3):
        blk = Z[:, :, FOFF[l]:FOFF[l] + 16 * LDIM[l]]
        blk = blk.reshape(NB, 128, 16, LDIM[l])
        nf = np.sum(blk.astype(np.float64) ** 2)
        out[:, :, 0, :, koff[l]:koff[l] + LDIM[l]] = blk / np.sqrt(nf / 16.0)
    return out
